# revision 1
# baseline (speedup 1.0000x reference)
"""Two-layer SimpleRNN (B=64, T=80, U=2048) on 8 TRN2 NeuronCores.

Tensor-parallel: each core owns a 256-column slice of Wh0/Wx1/Wh1 (plus the
matching slices of Wx0/b0/b1), keeps all weights resident in SBUF, and
computes its h-chunk each step in transposed layout
    z^T[n, b] = sum_k W[k, n] * h^T[k, b]
(stationary = weight k-tile, moving = h^T k-tile, PSUM-accumulated over k,
bf16 inputs / fp32 accumulate, per-partition bias + tanh fused on ScalarE).

The per-step all-gather of the [256, 64] h-chunks is done with direct
SBUF->SBUF remote DMA (`remote_dma_broadcast`, XOR-relative destinations) —
each step one gather carries h0(t) together with h1(t-1). Receiver-side slot
d holds the chunk of core (me XOR d); the weight k-tiles are permuted
per-core on the host to match, so the SPMD program needs no core-id logic.
Cross-die destinations (Δtpb bit 2) land with an extra XOR 2 (D2D links pair
SEngines diagonally), compensated in rdests. Arrival is signalled by remote
semaphore increments (+2 per sender per step); the waits are attached to the
first consuming matmul after Tile scheduling (Tile's scheduler cannot model
peer-incremented semaphores).

kernel(**inputs) takes the FULL unsharded inputs and returns the FULL output.
"""

import sys
import time

sys.path.insert(0, "/opt/trn_rl_repo")

import numpy as np
import ml_dtypes

import concourse.bass as bass
import concourse.mybir as mybir
import concourse.bacc as bacc
import concourse.tile as tile
import concourse.bass_utils as bass_utils
from concourse.tile_rust import add_dep_helper

B = 64          # batch
import os
T = int(os.environ.get("RNN_T", "80"))  # sequence length
E = 100         # embedding dim
EP = 128        # embedding dim padded to one partition tile
U = 2048        # hidden units
NC = 8          # cores
UC = U // NC    # hidden columns per core (256)
NT = UC // 128  # n-tiles per core (2)
SLOT = 2 * NT * B   # cols per gather slot: [h0|h1] x [nt2] x [B] = 256

FP = mybir.dt.float32
BF = mybir.dt.bfloat16
AF = mybir.ActivationFunctionType
bf16 = ml_dtypes.bfloat16

_compiled = None


def _rdest(d):
    """Relative dest for slot d: cross-die (bit2) dests land with an extra
    XOR 2 (diagonal D2D pairing), compensate here."""
    return (0, d ^ 2 if d >= 4 else d)


def _build():
    nc = bacc.Bacc("TRN2", target_bir_lowering=False, debug=False, num_devices=NC)

    xt_d = nc.dram_tensor("xt", [EP, T * B], BF, kind="ExternalInput")
    wx0_d = nc.dram_tensor("wx0", [EP, UC], BF, kind="ExternalInput")
    wh0_d = nc.dram_tensor("wh0", [128, NC * NT * NT * 128], BF, kind="ExternalInput")
    wx1_d = nc.dram_tensor("wx1", [128, NC * NT * NT * 128], BF, kind="ExternalInput")
    wh1_d = nc.dram_tensor("wh1", [128, NC * NT * NT * 128], BF, kind="ExternalInput")
    b0_d = nc.dram_tensor("b0", [128, NT], FP, kind="ExternalInput")
    b1_d = nc.dram_tensor("b1", [128, NT], FP, kind="ExternalInput")
    wo_d = nc.dram_tensor("wo", [128, NC * NT], BF, kind="ExternalInput")
    bo_d = nc.dram_tensor("bo", [128, 1], FP, kind="ExternalInput")
    out_d = nc.dram_tensor("out", [B, 1], FP, kind="ExternalOutput")

    remote_sem = nc.alloc_semaphore("remote_sem")
    local_sem = nc.alloc_semaphore("local_sem")

    # gather_nops[t]: PE no-op after trigger(t); gets the HW arrival wait
    # (remote_sem >= 14*(t+1)) post-scheduling. All matmuls consuming
    # gather(t) slots are edge-ordered after it on the PE FIFO.
    gather_nops = {}

    with tile.TileContext(nc) as tc:
        with (
            tc.tile_pool(name="const", bufs=1) as const,
            tc.tile_pool(name="state", bufs=1) as state,
            tc.tile_pool(name="chunk", bufs=4) as chunk_pool,
            tc.tile_pool(name="psum", bufs=2, space="PSUM") as psum_pool,
            tc.tile_pool(name="dram", bufs=1, space="DRAM") as dram_pool,
        ):
            # ---- rerun safety: clear arrival sem, then all-core barrier ----
            clr = nc.gpsimd.sem_clear(remote_sem)
            bar_in = dram_pool.tile([128, 1], FP)
            bar_out = dram_pool.tile([NC * 128, 1], FP, addr_space="Shared")
            bar = nc.gpsimd.collective_compute(
                "AllGather", mybir.AluOpType.bypass,
                replica_groups=[list(range(NC))],
                ins=[bar_in[:]], outs=[bar_out[:]],
            )
            add_dep_helper(bar.ins, clr.ins, reason="barrier after sem clear")

            # ---- constants ----
            xt = const.tile([EP, T * B], BF)
            wx0 = const.tile([EP, UC], BF)
            wh0 = const.tile([128, NC * NT * NT * 128], BF)
            wx1 = const.tile([128, NC * NT * NT * 128], BF)
            wh1 = const.tile([128, NC * NT * NT * 128], BF)
            b0 = const.tile([128, NT], FP)
            b1 = const.tile([128, NT], FP)
            wo = const.tile([128, NC * NT], BF)
            bo = const.tile([128, 1], FP)
            for sb_t, dr_t in [
                (xt, xt_d), (wx0, wx0_d), (wh0, wh0_d), (wx1, wx1_d),
                (wh1, wh1_d), (b0, b0_d), (b1, b1_d), (wo, wo_d), (bo, bo_d),
            ]:
                nc.sync.dma_start(sb_t[:], dr_t[:])

            # ---- gather buffers (double-buffered by step parity) ----
            # hg[p][:, d*SLOT + w*NT*B + nt2*B + b]: slot d = chunk of core
            # (me ^ d); w: 0=h0, 1=h1. Slot 0 (own chunk) is read from hc
            # directly and never written.
            hg = [state.tile([128, NC * SLOT], BF, name=f"hg{i}") for i in (0, 1)]
            m0 = nc.gpsimd.memset(hg[0][:], 0.0)
            m1 = nc.gpsimd.memset(hg[1][:], 0.0)
            # memsets must precede the barrier: a post-barrier peer send may
            # land in hg at any time.
            add_dep_helper(bar.ins, m0.ins, reason="hg0 zeroed before barrier")
            add_dep_helper(bar.ins, m1.ins, reason="hg1 zeroed before barrier")

            hc_init = state.tile([128, SLOT], BF)
            nc.gpsimd.memset(hc_init[:], 0.0)

            def h_mov(prev_hg, prev_hc, d, w, nt2):
                """Moving operand: h{w}^T k-subtile nt2 of hidden-block (me^d)."""
                if d == 0:
                    return prev_hc[:, (w * NT + nt2) * B:(w * NT + nt2 + 1) * B]
                return prev_hg[:, d * SLOT + (w * NT + nt2) * B:
                               d * SLOT + (w * NT + nt2 + 1) * B]

            def wslice(w_sb, d, nt2, nt):
                i = (d * NT + nt2) * NT + nt
                return w_sb[:, i * 128:(i + 1) * 128]

            def layer_mms(zp, gi, prev_hg, prev_hc, t=None):
                """One layer's matmuls. gi: gather index consumed (for waits).
                t given => layer0 (Wx0 x_t + Wh0 h0); else Wx1 h0 + Wh1 h1."""
                for nt in range(NT):
                    if t is not None:
                        nc.tensor.matmul(
                            zp[:, nt, :], wx0[:, nt * 128:(nt + 1) * 128],
                            xt[:, t * B:(t + 1) * B], start=True, stop=False)
                        pairs = [(wh0, 0)]
                    else:
                        pairs = [(wx1, 0), (wh1, 1)]
                    n_mm = len(pairs) * NC * NT
                    i = 0
                    for w_sb, w in pairs:
                        for d in range(NC):
                            for nt2 in range(NT):
                                i += 1
                                mm = nc.tensor.matmul(
                                    zp[:, nt, :], wslice(w_sb, d, nt2, nt),
                                    h_mov(prev_hg, prev_hc, d, w, nt2),
                                    start=(t is None and i == 1),
                                    stop=(i == n_mm),
                                )
                                if d > 0 and gi in gather_nops:
                                    add_dep_helper(mm.ins, gather_nops[gi].ins,
                                                   sync=False,
                                                   reason="consume after arrival nop")

            def send(hc_t, t):
                """Gather(t): broadcast my chunk into slot-d of peers (me^d)."""
                dst = hg[t % 2]
                for d in range(1, NC):
                    rdests = [None] * 8
                    rdests[d] = _rdest(d)
                    p = nc.gpsimd.remote_dma_broadcast(
                        dst[:, d * SLOT:(d + 1) * SLOT], hc_t[:],
                        remote_sem, local_sem, rdests=rdests)
                    if t == 0 and d == 1:
                        add_dep_helper(p.ins, bar.ins, reason="sends after barrier")
                trig = nc.gpsimd.trigger_dma(count=None)
                wnop = nc.tensor.nop(hint=f"arrival_wait_{t}", nofuse=True)
                add_dep_helper(wnop.ins, trig.ins, sync=True,
                               reason="arrival nop after trigger")
                gather_nops[t] = wnop

            hc_prev = hc_init
            for t in range(T):
                prev_hg = hg[(t - 1) % 2]
                hc_t = chunk_pool.tile([128, SLOT], BF, tag="hc")

                z0p = psum_pool.tile([128, NT, B], FP, tag="z0")
                layer_mms(z0p, t - 1, prev_hg, hc_prev, t=t)
                for nt in range(NT):
                    nc.scalar.activation(
                        hc_t[:, nt * B:(nt + 1) * B], z0p[:, nt, :],
                        AF.Tanh, bias=b0[:, nt:nt + 1])

                if t == 0:
                    nc.gpsimd.memset(hc_t[:, NT * B:2 * NT * B], 0.0)
                else:
                    z1p = psum_pool.tile([128, NT, B], FP, tag="z1")
                    layer_mms(z1p, t - 1, prev_hg, hc_prev)
                    for nt in range(NT):
                        nc.scalar.activation(
                            hc_t[:, (NT + nt) * B:(NT + nt + 1) * B], z1p[:, nt, :],
                            AF.Tanh, bias=b1[:, nt:nt + 1])

                send(hc_t, t)
                if t == 0:
                    hc_first = hc_t
                hc_prev = hc_t

            # final h1(T-1) + gather(T)
            hc_t = chunk_pool.tile([128, SLOT], BF, tag="hc")
            z1p = psum_pool.tile([128, NT, B], FP, tag="z1")
            layer_mms(z1p, T - 1, hg[(T - 1) % 2], hc_prev)
            for nt in range(NT):
                nc.scalar.activation(
                    hc_t[:, (NT + nt) * B:(NT + nt + 1) * B], z1p[:, nt, :],
                    AF.Tanh, bias=b1[:, nt:nt + 1])
            send(hc_t, T)
            hc_prev = hc_t

            # head: out[b] = sigmoid(sum_k h1[b,k] Wo[k] + bo) on every core
            op = psum_pool.tile([B, 1], FP, tag="head")
            i = 0
            for d in range(NC):
                for nt2 in range(NT):
                    i += 1
                    mm = nc.tensor.matmul(
                        op[:, :], h_mov(hg[T % 2], hc_prev, d, 1, nt2),
                        wo[:, d * NT + nt2:d * NT + nt2 + 1],
                        start=(i == 1), stop=(i == NC * NT))
                    if d > 0:
                        add_dep_helper(mm.ins, gather_nops[T].ins, sync=False,
                                       reason="head after arrival nop")
            out_sb = state.tile([B, 1], FP)
            act_o = nc.scalar.activation(out_sb[:], op[:], AF.Sigmoid, bias=bo[:B, :])
            nc.sync.dma_start(out_d[:], out_sb[:])

            if os.environ.get("RNN_DEBUG"):
                dbg_d = nc.dram_tensor("dbg", [128, (NC + 1) * SLOT], BF,
                                       kind="ExternalOutput")
                dd1 = nc.sync.dma_start(dbg_d[:, :NC * SLOT], hg[0][:])
                dd2 = nc.sync.dma_start(dbg_d[:, NC * SLOT:], hc_first[:])
                add_dep_helper(dd1.ins, act_o.ins, reason="dbg after all")
                add_dep_helper(dd2.ins, act_o.ins, reason="dbg after all")
                dbg2_d = nc.dram_tensor("dbg2", [128, (NC + 1) * SLOT], BF,
                                        kind="ExternalOutput")
                dd3 = nc.sync.dma_start(dbg2_d[:, :NC * SLOT], hg[T % 2][:])
                dd4 = nc.sync.dma_start(dbg2_d[:, NC * SLOT:], hc_prev[:])
                add_dep_helper(dd3.ins, act_o.ins, reason="dbg after all")
                add_dep_helper(dd4.ins, act_o.ins, reason="dbg after all")

    # ---- post-scheduling: attach arrival waits to first consumer per gather ----
    for gi, wnop in gather_nops.items():
        wnop.wait_op(remote_sem, 14 * (gi + 1), "sem-ge", check=False)

    nc.compile()
    return nc


def _shard_inputs(inputs, emb, Wx0, Wh0, b0, Wx1, Wh1, b1, Wo, bo):
    """Host-side: embed + transpose + per-core slicing/permutation."""
    x = emb[inputs][:, :T]               # [B, T, E]
    xt = np.ascontiguousarray(x.transpose(2, 1, 0)).reshape(E, T * B)
    xt_p = np.zeros((EP, T * B), bf16)
    xt_p[:E] = xt.astype(bf16)

    def ktile_perm(w, c):
        # [U, UC] col-slice -> [128, 8*2*2*128]; k-tile (d, nt2) holds rows of
        # hidden-block (c ^ d) so gathered slot layout matches.
        wc = w[:, c * UC:(c + 1) * UC].astype(bf16).reshape(NC, NT, 128, NT, 128)
        wp = wc[np.arange(NC) ^ c]                     # [d, nt2, p, nt, col]
        return np.ascontiguousarray(wp.transpose(2, 0, 1, 3, 4)).reshape(128, -1)

    wo_all = Wo[:, 0].astype(bf16).reshape(NC, NT, 128)

    in_maps = []
    for c in range(NC):
        wx0_c = np.zeros((EP, UC), bf16)
        wx0_c[:E] = Wx0[:, c * UC:(c + 1) * UC].astype(bf16)
        wo_p = wo_all[np.arange(NC) ^ c]               # [d, nt2, p]
        in_maps.append({
            "xt": xt_p,
            "wx0": wx0_c,
            "wh0": ktile_perm(Wh0, c),
            "wx1": ktile_perm(Wx1, c),
            "wh1": ktile_perm(Wh1, c),
            "b0": np.ascontiguousarray(
                b0[c * UC:(c + 1) * UC].reshape(NT, 128).T),
            "b1": np.ascontiguousarray(
                b1[c * UC:(c + 1) * UC].reshape(NT, 128).T),
            "wo": np.ascontiguousarray(wo_p.transpose(2, 0, 1)).reshape(128, -1),
            "bo": np.full((128, 1), bo[0], np.float32),
        })
    return in_maps


def _get_compiled():
    global _compiled
    if _compiled is None:
        _compiled = _build()
    return _compiled


def kernel(inputs, emb, Wx0, Wh0, b0, Wx1, Wh1, b1, Wo, bo, _trace=False,
           _tmpdir=None):
    nc = _get_compiled()
    in_maps = _shard_inputs(
        np.asarray(inputs), np.asarray(emb, np.float32),
        np.asarray(Wx0, np.float32), np.asarray(Wh0, np.float32),
        np.asarray(b0, np.float32), np.asarray(Wx1, np.float32),
        np.asarray(Wh1, np.float32), np.asarray(b1, np.float32),
        np.asarray(Wo, np.float32), np.asarray(bo, np.float32))
    res = bass_utils.run_bass_kernel_spmd(
        nc, in_maps, core_ids=list(range(NC)), trace=_trace, tmpdir=_tmpdir)
    out = res.results[0]["out"]
    if os.environ.get("RNN_DEBUG"):
        return out, res
    if _trace:
        return out, res
    return out


if __name__ == "__main__":
    t0 = time.time()
    _get_compiled()
    print(f"build+compile: {time.time()-t0:.1f}s")

